# revision 27
# baseline (speedup 1.0000x reference)
"""Chamfer distance kernel for Trainium2 (8 NeuronCores, batch-parallel).

Problem: xyz1, xyz2 of shape (8, 8192, 3) fp32. For each batch b:
  D[n, m] = ||xyz1[b,n] - xyz2[b,m]||^2  (squared distances)
  dist1[b, n] = min_m D[n, m]
  dist2[b, m] = min_n D[n, m]
Returns (dist1, dist2) both (8, 8192) fp32, matching reference.py.

Strategy (one batch per NeuronCore):
- Host packs each batch into augmented bf16 matrices A[24, N], B[24, M] such
  that (A^T @ B)[n, m] == -D[n, m] to ~fp32 accuracy. Each fp32 quantity is
  split into 3 bf16 components (hi/mid/lo); the inner product keeps all cross
  terms down to 2^-18 weight:
     -D = 2 x.y - ||x||^2 - ||y||^2
  Rows: per coord c: (2xh,yh) (2xh,ym) (2xm,yh) (2xh,yl) (2xl,yh) (2xm,ym)
  then (nxh,-1)(nxm,-1)(nxl,-1) and (1,-nyh)(1,-nym)(1,-nyl). K = 24.
- PE: for each 128-row n-chunk and 2048-wide m-group, 4 matmuls
  [K=24]x[128, 512] -> fp32 PSUM (negated distance tile).
- ACT (scalar engine): copies PSUM fp32 -> SBUF fp16 (conversion on write);
  the copy is routed into acc1/acc2 directly where it doubles as their init.
- DVE (vector engine): running elementwise max (fp16, 2x mode) into acc2
  (per m position, across n-chunks) and acc1 (per n row, across m-groups);
  after the last m-group a shrink + reduce_max emits the dist1 column.
  (tensor_tensor_reduce would fuse that but crashes TRN2 hardware.)
- PE transpose + DVE reduce collapse acc2's partition axis -> dist2.
- Host negates and reorders the two [128, chunks] outputs.

Measured ~655-670 us/iteration on HW (in-NEFF For_i loop differential),
DVE-bound at ~90% of its 2x-mode floor. Dead ends verified on HW/compiler:
tensor_tensor_reduce (device crash), GPSIMD tensor_tensor (walrus rejects
TT on Pool engine), nc.vector.pool_max (walrus rejects the lowered AP both
via Tile and raw - would have been the 4x-mode lead), fp32/PSUM-direct TT
(drops to 1x mode), matmul bf16 PSUM output (TRN3-only). The 2x-mode
tensor_tensor floor is the verified optimum reachable in this toolchain.
"""

import os
import sys

import numpy as np

try:
    import ml_dtypes
except ImportError:  # pragma: no cover
    sys.path.insert(0, "/opt/trn_rl_repo")
    import ml_dtypes

for _p in ("/root/.axon_site", "/root/.axon_site/_ro/trn_rl_repo", "/opt/trn_rl_repo"):
    if os.path.isdir(_p) and _p not in sys.path:
        sys.path.append(_p)

BF16 = ml_dtypes.bfloat16

_B, _N, _M = 8, 8192, 8192  # batches (= cores), points per cloud
_MG = 2048  # m-group width: 4 fp32 PSUM banks


# ----------------------------------------------------------------- host prep
def _split3(v32):
    """fp32 array -> (hi, mid, lo) bf16 arrays with hi+mid+lo ~= v to 2^-27."""
    h = v32.astype(BF16)
    r = v32 - h.astype(np.float32)
    m = r.astype(BF16)
    l = (r - m.astype(np.float32)).astype(BF16)
    return h, m, l


def _augment(xyz1, xyz2):
    """xyz1 [N,3], xyz2 [M,3] fp32 -> A [24,N] bf16, B [24,M] bf16 with
    (A^T @ B)[n, m] ~= -||x_n - y_m||^2."""
    x = np.ascontiguousarray(xyz1.T, dtype=np.float32)  # [3, N]
    y = np.ascontiguousarray(xyz2.T, dtype=np.float32)  # [3, M]
    xh, xm, xl = _split3(x)
    yh, ym, yl = _split3(y)

    def d2(a):  # exact doubling in bf16
        return (2.0 * a.astype(np.float32)).astype(BF16)

    nx = np.sum(x * x, axis=0, keepdims=True)  # [1, N] fp32
    ny = np.sum(y * y, axis=0, keepdims=True)  # [1, M]
    nxh, nxm, nxl = _split3(nx)
    nyh, nym, nyl = _split3(ny)

    N, M = x.shape[1], y.shape[1]
    ones_n = np.ones((1, N), dtype=BF16)
    ones_m = np.ones((1, M), dtype=BF16)

    # PE accumulates in row order: put the three large cancelling terms first
    # (2 xh.yh, -nxh, -nyh) so partial sums collapse to ~-D before the small
    # correction rows, minimizing fp32 accumulation error on tiny distances.
    a_rows = [d2(xh), -nxh, ones_n]
    b_rows = [yh, ones_m, -nyh]
    for ax, by in [(xh, ym), (xm, yh), (xh, yl), (xl, yh), (xm, ym)]:
        a_rows.append(d2(ax))
        b_rows.append(by)
    a_rows += [-nxm, -nxl, ones_n, ones_n]
    b_rows += [ones_m, ones_m, -nym, -nyl]

    A = np.concatenate(a_rows, axis=0)
    Bm = np.concatenate(b_rows, axis=0)
    assert A.shape[0] == 24 and Bm.shape[0] == 24
    return np.ascontiguousarray(A), np.ascontiguousarray(Bm)


# ------------------------------------------------------------- device program
def build_chamfer_nc(
    N=_N, M=_M, mg=_MG, repeat=1, loop=1, gp_shrink=False, pool_acc1=False
):
    """Build the single-core Bass program (SPMD across cores via run_*_spmd).

    repeat: python-unrolled repetitions of the whole body (benchmarking).
    loop: hardware For_i repetitions of the whole body (benchmarking).
    gp_shrink: run the dist1 shrink chains on GPSIMD instead of VectorE.
    pool_acc1: use pool_max(window 8) + mini-TT for the dist1 side (wins
        ~98 us/core iff HW pool runs in a 4x DVE mode; loses if 1x).
    """
    import contextlib

    import concourse.bacc as bacc
    import concourse.mybir as mybir
    import concourse.tile as tile

    F32 = mybir.dt.float32
    DBF16 = mybir.dt.bfloat16
    DF16 = mybir.dt.float16
    MAX = mybir.AluOpType.max
    X = mybir.AxisListType.X

    n_chunks = N // 128
    n_groups = M // mg
    sub = mg // 512

    nc = bacc.Bacc("TRN2", target_bir_lowering=False)
    A_d = nc.dram_tensor("A", [24, N], DBF16, kind="ExternalInput")
    B_d = nc.dram_tensor("B", [24, M], DBF16, kind="ExternalInput")
    I_d = nc.dram_tensor("I", [128, 128], DF16, kind="ExternalInput")
    ND1 = nc.dram_tensor("ND1", [128, n_chunks], F32, kind="ExternalOutput")
    ND2 = nc.dram_tensor("ND2", [128, M // 128], F32, kind="ExternalOutput")

    with tile.TileContext(nc) as tc:
        with tc.tile_pool(name="const", bufs=1) as const:
            A_sb = const.tile([24, N], DBF16)
            nc.sync.dma_start(A_sb[:], A_d[:])
            B_sb = const.tile([24, M], DBF16)
            nc.sync.dma_start(B_sb[:], B_d[:])
            I_sb = const.tile([128, 128], DF16)
            nc.sync.dma_start(I_sb[:], I_d[:])
            acc2 = const.tile([128, M], DF16)
            d1 = const.tile([128, n_chunks], F32)
            d2 = const.tile([128, M // 128], F32)
            # per-n-chunk shrunken dist1 partials, reduced once at the end
            stash_w = 256
            stash = const.tile([128, n_chunks * stash_w], DF16)

            loop_cm = tc.For_i(0, loop) if loop > 1 else contextlib.nullcontext()
            with loop_cm:
              for _rep in range(repeat):
                with (
                    tc.tile_pool(name="psum", bufs=2, space="PSUM") as psum_pool,
                    tc.tile_pool(name="cp", bufs=6) as cp_pool,
                    tc.tile_pool(name="acc1", bufs=2) as acc1_pool,
                    tc.tile_pool(name="shr", bufs=2) as shr_pool,
                ):
                    for nci in range(n_chunks):
                        lhsT = A_sb[:, nci * 128 : (nci + 1) * 128]
                        acc1 = acc1_pool.tile([128, mg], DF16)
                        if pool_acc1:
                            acc1m = acc1_pool.tile([128, stash_w], DF16, tag="a1m")
                        for g in range(n_groups):
                            pg = psum_pool.tile([128, mg], F32)
                            for s in range(sub):
                                nc.tensor.matmul(
                                    pg[:, s * 512 : (s + 1) * 512],
                                    lhsT,
                                    B_sb[:, g * mg + s * 512 : g * mg + (s + 1) * 512],
                                    start=True,
                                    stop=True,
                                )
                            gs = slice(g * mg, (g + 1) * mg)
                            # Route the ACT copy so it doubles as an
                            # accumulator init where possible.
                            if nci == 0:
                                # fresh tile straight into acc2 (its init)
                                nc.scalar.copy(acc2[:, gs], pg[:])
                                src = acc2[:, gs]
                                if g == 0 and n_groups > 1 and not pool_acc1:
                                    nc.vector.tensor_copy(acc1[:], src)
                            elif g == 0 and n_groups > 1 and not pool_acc1:
                                # fresh tile straight into acc1 (its init)
                                nc.scalar.copy(acc1[:], pg[:])
                                src = acc1[:]
                                nc.vector.tensor_max(acc2[:, gs], acc2[:, gs], src)
                            else:
                                cp = cp_pool.tile([128, mg], DF16)
                                nc.scalar.copy(cp[:], pg[:])
                                src = cp[:]
                                nc.vector.tensor_max(acc2[:, gs], acc2[:, gs], cp[:])
                            # dist1 side (tensor_tensor_reduce would fuse this
                            # but crashes TRN2 hardware; use TT + shrink tree)
                            if pool_acc1 and n_groups > 1:
                                ss = slice(nci * stash_w, (nci + 1) * stash_w)
                                w = mg // stash_w
                                srcv = src.rearrange(
                                    "p (a b c w) -> p a b c w",
                                    a=2, b=2, w=w,
                                )
                                if g == 0:
                                    nc.vector.pool_max(acc1m[:], srcv)
                                else:
                                    pm = shr_pool.tile(
                                        [128, stash_w], DF16, tag="pm"
                                    )
                                    nc.vector.pool_max(pm[:], srcv)
                                    dst = (
                                        stash[:, ss]
                                        if g == n_groups - 1
                                        else acc1m[:]
                                    )
                                    nc.vector.tensor_max(dst, acc1m[:], pm[:])
                            elif n_groups == 1:
                                nc.vector.reduce_max(
                                    d1[:, nci : nci + 1], src, axis=X
                                )
                            elif g == 0:
                                pass  # acc1 already initialized above
                            else:
                                nc.vector.tensor_max(acc1[:], acc1[:], src)
                                if g == n_groups - 1:
                                    # shrink mg -> stash_w by TT-max halving,
                                    # last level lands in the stash slice;
                                    # one batched reduce finishes dist1 later
                                    seng = nc.gpsimd if gp_shrink else nc.vector
                                    ss = slice(nci * stash_w, (nci + 1) * stash_w)
                                    w = mg // 2
                                    sh = shr_pool.tile([128, mg // 2], DF16)
                                    seng.tensor_max(
                                        sh[:, :w], acc1[:, :w], acc1[:, w:]
                                    )
                                    while w > 2 * stash_w:
                                        w //= 2
                                        seng.tensor_max(
                                            sh[:, :w], sh[:, :w], sh[:, w : 2 * w]
                                        )
                                    seng.tensor_max(
                                        stash[:, ss],
                                        sh[:, :stash_w],
                                        sh[:, stash_w : 2 * stash_w],
                                    )
                # dist1 final: one batched reduce over all stashed partials
                if n_groups > 1:
                    nc.vector.reduce_max(
                        d1[:],
                        stash[:].rearrange("p (c w) -> p c w", w=stash_w),
                        axis=X,
                    )
                # dist2 finals: transpose 128-blocks of acc2 into PSUM, 8
                # blocks per bank, then one batched reduce per bank
                with tc.tile_pool(name="tpsum", bufs=4, space="PSUM") as tp_pool:
                    for grp in range(M // 1024):
                        pt = tp_pool.tile([128, 1024], DF16)
                        for b in range(8):
                            blk = grp * 8 + b
                            nc.tensor.transpose(
                                pt[:, b * 128 : (b + 1) * 128],
                                acc2[:, blk * 128 : (blk + 1) * 128],
                                I_sb[:],
                            )
                        nc.vector.reduce_max(
                            d2[:, grp * 8 : (grp + 1) * 8],
                            pt[:].rearrange("p (c w) -> p c w", w=128),
                            axis=X,
                        )

            nc.sync.dma_start(ND1[:], d1[:])
            nc.sync.dma_start(ND2[:], d2[:])
    nc.finalize()
    return nc


# ------------------------------------------------------------------ execution
_RUNNER_CACHE = {}


def _make_runner(nc, n_cores):
    """Build a reusable jitted SPMD executor (mirrors bass2jax.run_bass_via_pjrt
    but keeps the jitted callable so repeat calls skip re-tracing)."""
    import jax
    import concourse.mybir as mybir
    from concourse import bass2jax
    from jax.sharding import Mesh, PartitionSpec
    from jax.experimental.shard_map import shard_map

    bass2jax.install_neuronx_cc_hook()

    partition_name = nc.partition_id_tensor.name if nc.partition_id_tensor else None
    in_names, out_names, out_avals, zero_outs = [], [], [], []
    for alloc in nc.m.functions[0].allocations:
        if not isinstance(alloc, mybir.MemoryLocationSet):
            continue
        name = alloc.memorylocations[0].name
        if alloc.kind == "ExternalInput":
            if name != partition_name:
                in_names.append(name)
        elif alloc.kind == "ExternalOutput":
            shape = tuple(alloc.tensor_shape)
            dtype = mybir.dt.np(alloc.dtype)
            out_names.append(name)
            out_avals.append(jax.core.ShapedArray(shape, dtype))
            zero_outs.append(np.zeros(shape, dtype))
    n_params = len(in_names)
    n_outs = len(out_avals)
    all_in_names = in_names + out_names
    if partition_name is not None:
        all_in_names.append(partition_name)
    donate = tuple(range(n_params, n_params + n_outs))

    def _body(*args):
        operands = list(args)
        if partition_name is not None:
            operands.append(bass2jax.partition_id_tensor())
        outs = bass2jax._bass_exec_p.bind(
            *operands,
            out_avals=tuple(out_avals),
            in_names=tuple(all_in_names),
            out_names=tuple(out_names),
            lowering_input_output_aliases=(),
            sim_require_finite=True,
            sim_require_nnan=True,
            nc=nc,
        )
        return tuple(outs)

    devices = jax.devices()[:n_cores]
    mesh = Mesh(np.asarray(devices), ("core",))
    sharded = jax.jit(
        shard_map(
            _body,
            mesh=mesh,
            in_specs=(PartitionSpec("core"),) * (n_params + n_outs),
            out_specs=(PartitionSpec("core"),) * n_outs,
            check_rep=False,
        ),
        donate_argnums=donate,
        keep_unused=True,
    )

    def run(in_maps):
        assert len(in_maps) == n_cores
        concat_in = [
            np.concatenate([np.asarray(m[name]) for m in in_maps], axis=0)
            for name in in_names
        ]
        concat_zeros = [
            np.zeros((n_cores * z.shape[0], *z.shape[1:]), z.dtype) for z in zero_outs
        ]
        out_arrs = sharded(*concat_in, *concat_zeros)
        out_np = [np.asarray(a) for a in out_arrs]
        return [
            {
                name: out_np[i].reshape(n_cores, *out_avals[i].shape)[c]
                for i, name in enumerate(out_names)
            }
            for c in range(n_cores)
        ]

    return run


def get_runner(repeat=1):
    key = ("runner", repeat)
    if key not in _RUNNER_CACHE:
        nc = build_chamfer_nc(repeat=repeat)
        _RUNNER_CACHE[key] = _make_runner(nc, _B)
    return _RUNNER_CACHE[key]


_IDENT = np.eye(128, dtype=np.float16)


def prep_in_maps(xyz1, xyz2):
    xyz1 = np.asarray(xyz1, dtype=np.float32)
    xyz2 = np.asarray(xyz2, dtype=np.float32)
    in_maps = []
    for b in range(xyz1.shape[0]):
        A, Bm = _augment(xyz1[b], xyz2[b])
        in_maps.append({"A": A, "B": Bm, "I": _IDENT})
    return in_maps


def postprocess(results):
    dist1 = np.stack([-r["ND1"].T.reshape(-1) for r in results])
    dist2 = np.stack([-r["ND2"].T.reshape(-1) for r in results])
    return dist1.astype(np.float32), dist2.astype(np.float32)


def kernel(xyz1, xyz2):
    run = get_runner()
    in_maps = prep_in_maps(xyz1, xyz2)
    results = run(in_maps)
    return postprocess(results)
